# revision 24
# baseline (speedup 1.0000x reference)
"""Trainium2 Bass kernel for nn_Attention_67370857005350.

Dense transformer block:
  q  = relu(pw_q  @ relu(bn(dwconv3x3(x))))            (2,512,64,64)
  kv = relu(pw_kv @ relu(bn(dwconv3x3_s2(features))))  (2,1024,32,32)
  out = relu(w_out @ softmax(q.k/8).v + b_out)         (2,256,64,64)

Sharding: spatial over query pixels — core c handles batch c//4, query
rows 16*(c%4) .. +16 (1024 q pixels).  Each core computes the full kv
branch for its batch (duplicated across the 4 cores of a batch; the kv
branch is ~12% of the FLOPs, and duplicating it removes every
collective).  No cross-core communication at all.

Per-core dataflow (all on-chip after the input DMAs):
  DVE:    3x3 depthwise convs as 9 scalar_tensor_tensor taps (q branch),
          relu epilogues, softmax normalize
  GPSIMD: kv-branch depthwise conv, partition-broadcast of 1/rowsum
  PE:     pointwise convs, q.k^T (transposed layout: kv on PSUM
          partitions), P@v via v^T produced directly by a row-parallel
          pointwise matmul (no PE transposes anywhere), to_out
  ACT:    exp (fused 1/8 scale), row-sum extraction copies

softmax is computed without the max-subtraction: dots = q.k/8 with
q,k >= 0 post-relu, and on this problem dots ∈ [0, 0.16], so exp is
safe in fp32 (softmax is shift-invariant so this matches the
reference's stabilized form).
"""

import os
import numpy as np

import concourse.bass as bass
import concourse.tile as tile
from concourse import bacc, mybir
from concourse.bass_utils import run_bass_kernel_spmd

# ---- problem constants (hardcoded; must match setup_inputs) ----
B = 2
DIM = 256            # input channels
INNER = 512          # q/k/v channels
HEADS = 8
D = INNER // HEADS   # 64 head dim
HW_ = 64             # image H = W
KVHW = 32            # kv image H = W after stride-2
NKV = KVHW * KVHW    # 1024 kv pixels per batch
N_CORES = 8
CORES_PER_BATCH = N_CORES // B
ROWS = HW_ // CORES_PER_BATCH   # 16 q rows per core
NQ = ROWS * HW_                 # 1024 q pixels per core
EPS = 1e-5
SCALE = float(D) ** -0.5        # 0.125

FP = mybir.dt.float32
FR = mybir.dt.float32r
BF = mybir.dt.bfloat16

# "f32r": fp32 storage, float32r matmuls (full-rate fp32-ish)
# "bf16": bf16 storage for matmul operands (weights pre-cast on host)
QUANT = os.environ.get("KERNEL_QUANT", "bf16")

AF = mybir.ActivationFunctionType
OP = mybir.AluOpType


def _mm(ap):
    return ap


def build_graph():
    """Build the SPMD graph (identical on all 8 cores)."""
    # dtype of matmul operands (DRAM weights / on-chip activations).
    # float32r is required end-to-end by the BIR verifier: producers of a
    # matmul operand must emit rounded-to-f32r values.
    w_dt = {"bf16": BF, "f32r": FR}.get(QUANT, FP)
    a_dt = w_dt

    nc = bacc.Bacc("TRN2", target_bir_lowering=False, debug=False,
                   enable_asserts=False)

    def din(name, shape, dt=FP):
        return nc.dram_tensor(name, shape, dt, kind="ExternalInput").ap()

    x_dt = BF if QUANT == "bf16" else FP  # host pre-casts images in bf16 mode
    # per-core shards (host pads/tranposes/folds; see kernel() below)
    xs = din("xs", [DIM, 18 * 66], x_dt)  # q-branch input rows, zero-padded
    fs = din("fs", [DIM, 66 * 66], x_dt)  # features (full batch), zero-padded
    dwq = din("dwq", [DIM, 9])            # BN-folded depthwise taps
    tqb = din("tqb", [DIM, 1])            # BN-folded bias
    dwk = din("dwk", [DIM, 9])
    tkb = din("tkb", [DIM, 1])
    pwqT = din("pwqT", [DIM, INNER], w_dt)    # lhsT for q pointwise
    pwkT = din("pwkT", [DIM, INNER], w_dt)    # lhsT for k pointwise
    wvT = din("wvT", [DIM, INNER], w_dt)      # rhs for v^T row-parallel pw
    woutT = din("woutT", [INNER, DIM], w_dt)  # lhsT for to_out
    bout = din("bout", [DIM, 1])
    if QUANT == "bf16":
        # per-(channel,tap) diagonal matrices for the q-branch depthwise
        # conv as PE matmuls (host-built; see _prep_shards)
        dgq = din("dgq", [DIM, 9 * 128], BF)
        dgq_r = dgq.rearrange("(t p) (k m) -> t p k m", p=128, k=9)
        dgk = din("dgk", [DIM, 9 * 128], BF)
        dgk_r = dgk.rearrange("(t p) (k m) -> t p k m", p=128, k=9)
    out = nc.dram_tensor("out", [DIM, NQ], FP, kind="ExternalOutput").ap()

    xs_r = xs.rearrange("(t p) (a b) -> t p a b", p=128, a=18)
    fs_r = fs.rearrange("(t p) (a b) -> t p a b", p=128, a=66)
    dwq_r = dwq.rearrange("(t p) k -> t p k", p=128)
    dwk_r = dwk.rearrange("(t p) k -> t p k", p=128)
    tqb_r = tqb.rearrange("(t p) k -> t p k", p=128)
    tkb_r = tkb.rearrange("(t p) k -> t p k", p=128)
    pwqT_r = pwqT.rearrange("(t p) n -> t p n", p=128)
    pwkT_r = pwkT.rearrange("(t p) n -> t p n", p=128)
    wvT_r = wvT.rearrange("(t p) n -> t p n", p=128)
    woutT_r = woutT.rearrange("(t p) n -> t p n", p=128)
    bout_r = bout.rearrange("(t p) k -> t p k", p=128)
    out_r = out.rearrange("(t p) n -> t p n", p=128)

    with tile.TileContext(nc) as tc:
        with (
            tc.tile_pool(name="const", bufs=1) as cpool,
            tc.tile_pool(name="inbuf", bufs=1) as inpool,
            tc.tile_pool(name="acc", bufs=2) as accpool,
            tc.tile_pool(name="act", bufs=1) as actpool,
            tc.tile_pool(name="exp", bufs=4) as epool,
            tc.tile_pool(name="small", bufs=2) as spool,
            tc.tile_pool(name="usbp", bufs=3) as uspool,
            tc.tile_pool(name="ps", bufs=2, space="PSUM") as ps,
            tc.tile_pool(name="psu", bufs=2, space="PSUM") as psu,
        ):
            # ---------------- input DMAs ----------------
            # Three parallel DMA paths (SP-HWDGE, ACT-HWDGE, Pool-SWDGE),
            # ordered so the tensors that gate compute arrive first:
            #   sync:   dgq + x slice + q-branch weights  (PE dw-q matmuls)
            #   scalar: kv tap weights + features ct0     (DVE kv taps)
            #   gpsimd: features ct1 + remaining weights
            xp = inpool.tile([128, 2, 18, 66], x_dt)
            fp = inpool.tile([128, 2, 66, 66], x_dt)
            dwq_sb = cpool.tile([128, 2, 9], FP)
            dwk_sb = cpool.tile([128, 2, 9], FP)
            tqb_sb = cpool.tile([128, 2, 1], FP)
            tkb_sb = cpool.tile([128, 2, 1], FP)
            pwqT_sb = cpool.tile([128, 2, INNER], w_dt)
            pwkT_sb = cpool.tile([128, 2, INNER], w_dt)
            wvT_sb = cpool.tile([128, 2, INNER], w_dt)
            woutT_sb = cpool.tile([128, 4, DIM], w_dt)
            bout_sb = cpool.tile([128, 2, 1], FP)
            if QUANT == "bf16":
                dgq_sb = cpool.tile([128, 2, 9, 128], BF)
                nc.sync.dma_start(
                    dgq_sb[:, :, :, :],
                    dgq_r.rearrange("t p k m -> p t k m"))
                dgk_sb = cpool.tile([128, 2, 9, 128], BF)
                nc.scalar.dma_start(
                    dgk_sb[:, :, :, :],
                    dgk_r.rearrange("t p k m -> p t k m"))
            for t in range(2):
                nc.scalar.dma_start(dwk_sb[:, t, :], dwk_r[t])
                nc.scalar.dma_start(tkb_sb[:, t, :], tkb_r[t])
            nc.scalar.dma_start(fp[:, 0, :, :], fs_r[0])
            nc.gpsimd.dma_start(fp[:, 1, :, :], fs_r[1])
            nc.sync.dma_start(dwq_sb[:, :, :],
                              dwq_r.rearrange("t p k -> p t k"))
            nc.sync.dma_start(tqb_sb[:, :, :],
                              tqb_r.rearrange("t p k -> p t k"))
            nc.sync.dma_start(xp[:, :, :, :],
                              xs_r.rearrange("t p a b -> p t a b"))
            for t in range(2):
                nc.sync.dma_start(pwqT_sb[:, t, :], pwqT_r[t])
                nc.scalar.dma_start(pwkT_sb[:, t, :], pwkT_r[t])
                nc.gpsimd.dma_start(wvT_sb[:, t, :], wvT_r[t])
                nc.gpsimd.dma_start(bout_sb[:, t, :], bout_r[t])
            for t in range(4):
                nc.gpsimd.dma_start(woutT_sb[:, t, :], woutT_r[t])

            # v^T staging: [kv-chunk, head, 66] blocks; col 64 of each block
            # is the ones column (row-sum trick), col 65 unused padding.
            # (memset doesn't support f32r, so copy from an f32 ones tile.)
            vt_sb = actpool.tile([128, 8, HEADS, 66], a_dt)
            ones_sb = cpool.tile([128, 64], FP)
            nc.gpsimd.memset(ones_sb[:, :], 1.0)
            nc.vector.tensor_copy(
                vt_sb[:, :, :, 64:65],
                ones_sb[:, :].rearrange("p (a b c) -> p a b c", a=8, b=HEADS))

            tq = actpool.tile([128, 2, NQ], a_dt)
            tkv = actpool.tile([128, 2, NKV], a_dt)

            # ---------------- depthwise convs ----------------
            # All taps on DVE: GPSIMD's Pool ISA has no TensorScalarPtr
            # (per-partition scalar) op.  kv branch first — k/v gate more
            # PE work than q.
            def dw_conv(eng, src_ap, stride, n, wtile, btile, ct, dst,
                        half=None, epi_eng=None):
                # half: process only pixel rows [half] (kv branch) so the
                # first half of k/v unblocks attention chunks 0-3 early.
                acc = accpool.tile([128, n], FP, tag="dwacc")
                rows = 16 if stride == 1 else 16
                r0 = 0 if not half else (32 if stride == 1 else 32)
                av = acc[:, :].rearrange("p (a b) -> p a b", a=rows)
                for tap in range(9):
                    dy, dx = tap // 3, tap % 3
                    if stride == 1:
                        s = src_ap[:, ct, dy:dy + 16, dx:dx + 64]
                    else:
                        y0 = dy + half * 32
                        s = src_ap[:, ct, y0:y0 + 32:2, dx:dx + 64:2]
                    w = wtile[:, ct, tap:tap + 1]
                    if tap == 0:
                        eng.tensor_scalar(av, s, w, None, op0=OP.mult)
                    else:
                        eng.scalar_tensor_tensor(av, s, w, av,
                                                 op0=OP.mult, op1=OP.add)
                # t = relu(acc + bias); output dtype = a_dt
                if epi_eng is nc.scalar:
                    nc.scalar.activation(dst, acc[:, :], AF.Relu,
                                         bias=btile[:, ct, :])
                else:
                    nc.vector.tensor_scalar(dst, acc[:, :], btile[:, ct, :],
                                            0.0, op0=OP.add, op1=OP.max)

            def dwq_pe(ct):
                acc = psu.tile([128, 1024], FP, tag="uR")
                for half in range(2):
                    o = acc[:, half * 512:(half + 1) * 512]
                    for tap in range(9):
                        dy, dx = tap // 3, tap % 3
                        r0 = half * 8
                        rhs = xp[:, ct, dy + r0:dy + r0 + 8, dx:dx + 64]
                        nc.tensor.matmul(
                            o, dgq_sb[:, ct, tap, :], rhs,
                            start=(tap == 0), stop=(tap == 8))
                nc.scalar.activation(tq[:, ct, :], acc[:, :], AF.Relu,
                                     bias=tqb_sb[:, ct, :])

            def dwk_pe(ct, half):
                acc = psu.tile([128, 512], FP, tag="uR",
                               name=f"dwkacc_{ct}_{half}")
                for tap in range(9):
                    dy, dx = tap // 3, tap % 3
                    y0 = dy + half * 32
                    rhs = fp[:, ct, y0:y0 + 32:2, dx:dx + 64:2]
                    nc.tensor.matmul(acc[:, :], dgk_sb[:, ct, tap, :], rhs,
                                     start=(tap == 0), stop=(tap == 8))
                nc.scalar.activation(
                    tkv[:, ct, half * 512:(half + 1) * 512], acc[:, :],
                    AF.Relu, bias=tkb_sb[:, ct, :])

            q_sb = actpool.tile([128, 4, NQ], a_dt)
            k_sb = actpool.tile([128, 4, NKV], a_dt)

            def relu_epi(eng, out, in_):
                # relu from PSUM; on ScalarE (idle pre-attention, and relu
                # shares exp's ACT table set) or DVE (slack mid-attention)
                if eng is nc.scalar:
                    nc.scalar.activation(out, in_, AF.Relu)
                else:
                    eng.tensor_scalar(out, in_, 0.0, None, op0=OP.max)

            def pw_k_half(half, epi_eng):
                # k: [kc on partitions, kv pixels]  (column-parallel)
                for mt in range(4):
                    pk = ps.tile([128, 512], FP, tag="mm")
                    for ct in range(2):
                        nc.tensor.matmul(
                            pk[:, :],
                            _mm(pwkT_sb[:, ct, mt * 128:(mt + 1) * 128]),
                            _mm(tkv[:, ct, half * 512:(half + 1) * 512]),
                            start=(ct == 0), stop=(ct == 1))
                    relu_epi(epi_eng,
                             k_sb[:, mt, half * 512:(half + 1) * 512],
                             pk[:, :])

            def pw_v_range(kts, epi_eng):
                # v^T: [kv pixels on partitions, vc]  (row-parallel)
                for kt in kts:
                    pv = ps.tile([128, 1024], FP, tag="mm")
                    for ct in range(2):
                        nc.tensor.matmul(
                            pv[:, 0:512],
                            _mm(tkv[:, ct, kt * 128:(kt + 1) * 128]),
                            _mm(wvT_sb[:, ct, :]),
                            start=(ct == 0), stop=(ct == 1))
                    relu_epi(epi_eng, vt_sb[:, kt, :, 0:64],
                             pv[:, 0:512].rearrange("p (h d) -> p h d",
                                                    h=HEADS))

            def pw_q_all():
                # q: [qc on partitions, q pixels]
                for mt in range(4):
                    pq = ps.tile([128, 1024], FP, tag="mm")
                    for half in range(2):
                        o = pq[:, half * 512:(half + 1) * 512]
                        for ct in range(2):
                            nc.tensor.matmul(
                                o,
                                _mm(pwqT_sb[:, ct, mt * 128:(mt + 1) * 128]),
                                _mm(tq[:, ct, half * 512:(half + 1) * 512]),
                                start=(ct == 0), stop=(ct == 1))
                    relu_epi(nc.scalar, q_sb[:, mt, :], pq[:, :])

            # Emission order tuned for overlap: the DVE kv-tap chains are
            # the long serial pole at the start, so they lead; PE picks up
            # each downstream matmul group as its inputs land.
            if QUANT == "bf16":
                for ct in range(2):
                    dwk_pe(ct, 0)
                for ct in range(2):
                    dwq_pe(ct)
                for ct in range(2):
                    dwk_pe(ct, 1)
            else:
                for ct in range(2):
                    dw_conv(nc.vector, fp, 2, NKV // 2, dwk_sb, tkb_sb, ct,
                            tkv[:, ct, 0:512], half=0, epi_eng=nc.scalar)
                for ct in range(2):
                    dw_conv(nc.vector, xp, 1, NQ, dwq_sb, tqb_sb, ct,
                            tq[:, ct, :], epi_eng=nc.scalar)
                for ct in range(2):
                    dw_conv(nc.vector, fp, 2, NKV // 2, dwk_sb, tkb_sb, ct,
                            tkv[:, ct, 512:1024], half=1, epi_eng=nc.vector)
            pw_q_all()
            pw_k_half(0, nc.scalar)
            pw_v_range(range(0, 4), nc.scalar)
            pw_k_half(1, nc.vector)
            pw_v_range(range(4, 8), nc.vector)

            # ---------------- attention ----------------
            # Heads processed in PAIRS with interleaved kv chunks: chunks
            # 0-3 of both heads only need the first tkv half, so they
            # overlap the DVE tap chains producing the second half.
            # After a head's P@v accumulation, uR is copied to SBUF at once
            # (frees its PSUM slot ~4us earlier than waiting for the whole
            # normalize chain), and to_out's K-accumulation is folded in
            # per pair (att rows of pair hp are exactly K-chunk hp).
            att_sb = actpool.tile([128, 4, NQ], a_dt)

            def normalize(h, u_sb, rrow):
                # att = u * (1/rowsum); rowsum = row 64 (ones-column trick).
                # (reciprocal_approx_fast only from partition 0 — reading it
                # at base partition 64 wedged the exec unit.)
                po = (h % 2) * 64
                pt = h // 2
                invr = spool.tile([1, 1024], FP, tag="invr")
                nc.vector.reciprocal_approx_fast(invr[:, :], rrow[:, :])
                invrb = spool.tile([64, 1024], FP, tag="invrb")
                nc.gpsimd.partition_broadcast(invrb[:, :], invr[:, :])
                nc.vector.tensor_tensor(att_sb[po:po + 64, pt, :],
                                        u_sb[0:64, :], invrb[:, :],
                                        op=OP.mult)

            for hp in range(HEADS // 2):
                heads = (2 * hp, 2 * hp + 1)
                uRs = [psu.tile([65, 1024], FP, tag="uR",
                                name=f"uR_{hp}_{j}") for j in range(2)]
                pend = [[], []]
                for c in range(8):
                    for j, h in enumerate(heads):
                        po = (h % 2) * 64
                        pt = h // 2
                        dp = ps.tile([128, 1024], FP, tag="mm")
                        for half in range(2):
                            nc.tensor.matmul(
                                dp[:, half * 512:(half + 1) * 512],
                                _mm(k_sb[po:po + 64, pt,
                                         c * 128:(c + 1) * 128]),
                                _mm(q_sb[po:po + 64, pt,
                                         half * 512:(half + 1) * 512]),
                                start=True, stop=True)
                        e = epool.tile([128, 1024], a_dt, tag="e")
                        nc.scalar.activation(e[:, :], dp[:, :], AF.Exp,
                                             scale=SCALE)
                        pend[j].append((c, e))
                        if len(pend[j]) > 1:
                            _emit_pv(nc, uRs[j], vt_sb, pend[j].pop(0), h)
                for j, h in enumerate(heads):
                    _emit_pv(nc, uRs[j], vt_sb, pend[j].pop(0), h)
                    rrow = spool.tile([1, 1024], FP, tag="rrow",
                                      name=f"rrow_{hp}_{j}")
                    nc.vector.tensor_copy(rrow[:, :], uRs[j][64:65, :])
                    if hp < HEADS // 2 - 1:
                        u_sb = uspool.tile([64, 1024], FP, tag="usb",
                                           name=f"usb_{hp}_{j}")
                        nc.vector.tensor_copy(u_sb[:, :], uRs[j][0:64, :])
                        normalize(h, u_sb, rrow)
                    else:
                        normalize(h, uRs[j], rrow)

            # ---------------- to_out ----------------
            osb = actpool.tile([128, 2, NQ], FP)
            for mt in range(2):
                pso = ps.tile([128, 1024], FP, tag="mm")
                for half in range(2):
                    o = pso[:, half * 512:(half + 1) * 512]
                    for ct in range(4):
                        nc.tensor.matmul(
                            o, _mm(woutT_sb[:, ct, mt * 128:(mt + 1) * 128]),
                            _mm(att_sb[:, ct, half * 512:(half + 1) * 512]),
                            start=(ct == 0), stop=(ct == 3))
                nc.vector.tensor_scalar(osb[:, mt, :], pso[:, :],
                                        bout_sb[:, mt, :], 0.0,
                                        op0=OP.add, op1=OP.max)
                nc.sync.dma_start(out_r[mt], osb[:, mt, :])

    nc.compile()
    return nc


def _emit_pv(nc, uR, vt_sb, ce, h):
    c, e = ce
    for half in range(2):
        nc.tensor.matmul(uR[:, half * 512:(half + 1) * 512],
                         _mm(vt_sb[:, c, h, 0:65]),
                         _mm(e[:, half * 512:(half + 1) * 512]),
                         start=(c == 0), stop=(c == 7))


_NC_CACHE = {}


def _get_nc():
    key = QUANT
    if key not in _NC_CACHE:
        _NC_CACHE[key] = build_graph()
    return _NC_CACHE[key]


def _prep_shards(inputs):
    """Host-side sharding/layout prep. Returns in_maps for the 8 cores."""
    f32 = lambda a: np.ascontiguousarray(np.asarray(a, np.float32))
    w_np = np.float32 if QUANT != "bf16" else None

    def wcast(a):
        a = np.ascontiguousarray(np.asarray(a, np.float32))
        if QUANT == "bf16":
            import ml_dtypes
            a = a.astype(ml_dtypes.bfloat16)
        return a

    x = f32(inputs["x"])
    features = f32(inputs["features"])

    # fold BN into depthwise weights/bias
    sq = f32(inputs["bnq_g"]) / np.sqrt(f32(inputs["bnq_v"]) + EPS)
    sk = f32(inputs["bnk_g"]) / np.sqrt(f32(inputs["bnk_v"]) + EPS)
    dwq = f32(inputs["dw_q"])[:, 0] * sq[:, None, None]
    dwk = f32(inputs["dw_kv"])[:, 0] * sk[:, None, None]
    dwq = np.ascontiguousarray(dwq.reshape(DIM, 9))
    dwk = np.ascontiguousarray(dwk.reshape(DIM, 9))
    tqb = np.ascontiguousarray(
        (f32(inputs["bnq_b"]) - f32(inputs["bnq_m"]) * sq).reshape(DIM, 1))
    tkb = np.ascontiguousarray(
        (f32(inputs["bnk_b"]) - f32(inputs["bnk_m"]) * sk).reshape(DIM, 1))

    pw_q = f32(inputs["pw_q"])[:, :, 0, 0]       # (512, 256)
    pw_kv = f32(inputs["pw_kv"])[:, :, 0, 0]     # (1024, 256)
    w_out = f32(inputs["w_out"])[:, :, 0, 0]     # (256, 512)
    pwqT = wcast(pw_q.T)                          # (256, 512)
    pwkT = wcast(pw_kv[:INNER].T)                 # (256, 512)
    wvT = wcast(pw_kv[INNER:].T)                  # (256, 512)
    woutT = wcast(w_out.T)                        # (512, 256)
    bout = np.ascontiguousarray(f32(inputs["b_out"]).reshape(DIM, 1))

    dgq = dgk = None
    if QUANT == "bf16":
        # diagonal per-tap matrices for the PE q-branch depthwise conv
        import ml_dtypes
        cc = np.arange(DIM)
        def diag(w):
            d = np.zeros((DIM, 9, 128), np.float32)
            d[cc, :, cc % 128] = w
            return np.ascontiguousarray(
                d.reshape(DIM, 9 * 128).astype(ml_dtypes.bfloat16))
        dgq = diag(dwq)
        dgk = diag(dwk)

    # zero-padded images
    xpad = np.zeros((B, DIM, HW_ + 2, HW_ + 2), np.float32)
    xpad[:, :, 1:-1, 1:-1] = x
    fpad = np.zeros((B, DIM, HW_ + 2, HW_ + 2), np.float32)
    fpad[:, :, 1:-1, 1:-1] = features

    if QUANT == "bf16":
        # images are stored/DMA'd in bf16 (the kernel computes in bf16)
        import ml_dtypes
        xpad = xpad.astype(ml_dtypes.bfloat16)
        fpad = fpad.astype(ml_dtypes.bfloat16)

    in_maps = []
    for c in range(N_CORES):
        b = c // CORES_PER_BATCH
        r0 = (c % CORES_PER_BATCH) * ROWS
        xs_c = np.ascontiguousarray(
            xpad[b, :, r0:r0 + ROWS + 2, :].reshape(DIM, 18 * 66))
        fs_c = np.ascontiguousarray(fpad[b].reshape(DIM, 66 * 66))
        m = {
            "xs": xs_c, "fs": fs_c,
            "dwq": dwq, "tqb": tqb, "dwk": dwk, "tkb": tkb,
            "pwqT": pwqT, "pwkT": pwkT, "wvT": wvT,
            "woutT": woutT, "bout": bout,
        }
        if dgq is not None:
            m["dgq"] = dgq
            m["dgk"] = dgk
        in_maps.append(m)
    return in_maps


def kernel(**inputs):
    nc = _get_nc()
    in_maps = _prep_shards(inputs)
    trace = os.environ.get("KERNEL_TRACE", "0") == "1"
    res = run_bass_kernel_spmd(nc, in_maps, core_ids=list(range(N_CORES)),
                               trace=trace)
    if trace:
        kernel.last_exec_time_ns = res.exec_time_ns
        kernel.last_results = res
    out = np.zeros((B, DIM, HW_, HW_), np.float32)
    for c in range(N_CORES):
        b = c // CORES_PER_BATCH
        r0 = (c % CORES_PER_BATCH) * ROWS
        out[b, :, r0:r0 + ROWS, :] = res.results[c]["out"].reshape(
            DIM, ROWS, HW_)
    return out


if __name__ == "__main__":
    nc = build_graph()
    print("graph built + compiled OK")


# revision 26
# speedup vs baseline: 1.0312x; 1.0312x over previous
"""Trainium2 Bass kernel for nn_Attention_67370857005350.

Dense transformer block:
  q  = relu(pw_q  @ relu(bn(dwconv3x3(x))))            (2,512,64,64)
  kv = relu(pw_kv @ relu(bn(dwconv3x3_s2(features))))  (2,1024,32,32)
  out = relu(w_out @ softmax(q.k/8).v + b_out)         (2,256,64,64)

Sharding: spatial over query pixels — core c handles batch c//4, query
rows 16*(c%4) .. +16 (1024 q pixels).  Each core computes the full kv
branch for its batch (duplicated across the 4 cores of a batch; the kv
branch is ~12% of the FLOPs, and duplicating it removes every
collective).  No cross-core communication at all.

Per-core dataflow (all on-chip after the input DMAs):
  DVE:    3x3 depthwise convs as 9 scalar_tensor_tensor taps (q branch),
          relu epilogues, softmax normalize
  GPSIMD: kv-branch depthwise conv, partition-broadcast of 1/rowsum
  PE:     pointwise convs, q.k^T (transposed layout: kv on PSUM
          partitions), P@v via v^T produced directly by a row-parallel
          pointwise matmul (no PE transposes anywhere), to_out
  ACT:    exp (fused 1/8 scale), row-sum extraction copies

softmax is computed without the max-subtraction: dots = q.k/8 with
q,k >= 0 post-relu, and on this problem dots ∈ [0, 0.16], so exp is
safe in fp32 (softmax is shift-invariant so this matches the
reference's stabilized form).
"""

import os
import numpy as np

import concourse.bass as bass
import concourse.tile as tile
from concourse import bacc, mybir
from concourse.bass_utils import run_bass_kernel_spmd

# ---- problem constants (hardcoded; must match setup_inputs) ----
B = 2
DIM = 256            # input channels
INNER = 512          # q/k/v channels
HEADS = 8
D = INNER // HEADS   # 64 head dim
HW_ = 64             # image H = W
KVHW = 32            # kv image H = W after stride-2
NKV = KVHW * KVHW    # 1024 kv pixels per batch
N_CORES = 8
CORES_PER_BATCH = N_CORES // B
ROWS = HW_ // CORES_PER_BATCH   # 16 q rows per core
NQ = ROWS * HW_                 # 1024 q pixels per core
EPS = 1e-5
SCALE = float(D) ** -0.5        # 0.125

FP = mybir.dt.float32
FR = mybir.dt.float32r
BF = mybir.dt.bfloat16

# "f32r": fp32 storage, float32r matmuls (full-rate fp32-ish)
# "bf16": bf16 storage for matmul operands (weights pre-cast on host)
QUANT = os.environ.get("KERNEL_QUANT", "bf16")

AF = mybir.ActivationFunctionType
OP = mybir.AluOpType


def _mm(ap):
    return ap


def build_graph():
    """Build the SPMD graph (identical on all 8 cores)."""
    # dtype of matmul operands (DRAM weights / on-chip activations).
    # float32r is required end-to-end by the BIR verifier: producers of a
    # matmul operand must emit rounded-to-f32r values.
    w_dt = {"bf16": BF, "f32r": FR}.get(QUANT, FP)
    a_dt = w_dt

    nc = bacc.Bacc("TRN2", target_bir_lowering=False, debug=False,
                   enable_asserts=False)

    def din(name, shape, dt=FP):
        return nc.dram_tensor(name, shape, dt, kind="ExternalInput").ap()

    x_dt = BF if QUANT == "bf16" else FP  # host pre-casts images in bf16 mode
    # per-core shards (host pads/tranposes/folds; see kernel() below)
    xs = din("xs", [DIM, 18 * 66], x_dt)  # q-branch input rows, zero-padded
    fs = din("fs", [DIM, 66 * 66], x_dt)  # features (full batch), zero-padded
    dwq = din("dwq", [DIM, 9])            # BN-folded depthwise taps
    tqb = din("tqb", [DIM, 1])            # BN-folded bias
    dwk = din("dwk", [DIM, 9])
    tkb = din("tkb", [DIM, 1])
    pwqT = din("pwqT", [DIM, INNER], w_dt)    # lhsT for q pointwise
    pwkT = din("pwkT", [DIM, INNER], w_dt)    # lhsT for k pointwise
    wvT = din("wvT", [DIM, INNER], w_dt)      # rhs for v^T row-parallel pw
    woutT = din("woutT", [INNER, DIM], w_dt)  # lhsT for to_out
    bout = din("bout", [DIM, 1])
    if QUANT == "bf16":
        # per-(channel,tap) diagonal matrices for the q-branch depthwise
        # conv as PE matmuls (host-built; see _prep_shards)
        dgq = din("dgq", [DIM, 9 * 128], BF)
        dgq_r = dgq.rearrange("(t p) (k m) -> t p k m", p=128, k=9)
    out = nc.dram_tensor("out", [DIM, NQ], FP, kind="ExternalOutput").ap()

    xs_r = xs.rearrange("(t p) (a b) -> t p a b", p=128, a=18)
    fs_r = fs.rearrange("(t p) (a b) -> t p a b", p=128, a=66)
    dwq_r = dwq.rearrange("(t p) k -> t p k", p=128)
    dwk_r = dwk.rearrange("(t p) k -> t p k", p=128)
    tqb_r = tqb.rearrange("(t p) k -> t p k", p=128)
    tkb_r = tkb.rearrange("(t p) k -> t p k", p=128)
    pwqT_r = pwqT.rearrange("(t p) n -> t p n", p=128)
    pwkT_r = pwkT.rearrange("(t p) n -> t p n", p=128)
    wvT_r = wvT.rearrange("(t p) n -> t p n", p=128)
    woutT_r = woutT.rearrange("(t p) n -> t p n", p=128)
    bout_r = bout.rearrange("(t p) k -> t p k", p=128)
    out_r = out.rearrange("(t p) n -> t p n", p=128)

    with tile.TileContext(nc) as tc:
        with (
            tc.tile_pool(name="const", bufs=1) as cpool,
            tc.tile_pool(name="inbuf", bufs=1) as inpool,
            tc.tile_pool(name="acc", bufs=2) as accpool,
            tc.tile_pool(name="act", bufs=1) as actpool,
            tc.tile_pool(name="exp", bufs=4) as epool,
            tc.tile_pool(name="small", bufs=2) as spool,
            tc.tile_pool(name="usbp", bufs=3) as uspool,
            tc.tile_pool(name="ps", bufs=2, space="PSUM") as ps,
            tc.tile_pool(name="psu", bufs=2, space="PSUM") as psu,
        ):
            # ---------------- input DMAs ----------------
            # Three parallel DMA paths (SP-HWDGE, ACT-HWDGE, Pool-SWDGE),
            # ordered so the tensors that gate compute arrive first:
            #   sync:   dgq + x slice + q-branch weights  (PE dw-q matmuls)
            #   scalar: kv tap weights + features ct0     (DVE kv taps)
            #   gpsimd: features ct1 + remaining weights
            xp = inpool.tile([128, 2, 18, 66], x_dt)
            fp = inpool.tile([128, 2, 66, 66], x_dt)
            dwq_sb = cpool.tile([128, 2, 9], FP)
            dwk_sb = cpool.tile([128, 2, 9], FP)
            tqb_sb = cpool.tile([128, 2, 1], FP)
            tkb_sb = cpool.tile([128, 2, 1], FP)
            pwqT_sb = cpool.tile([128, 2, INNER], w_dt)
            pwkT_sb = cpool.tile([128, 2, INNER], w_dt)
            wvT_sb = cpool.tile([128, 2, INNER], w_dt)
            woutT_sb = cpool.tile([128, 4, DIM], w_dt)
            bout_sb = cpool.tile([128, 2, 1], FP)
            if QUANT == "bf16":
                dgq_sb = cpool.tile([128, 2, 9, 128], BF)
                nc.sync.dma_start(
                    dgq_sb[:, :, :, :],
                    dgq_r.rearrange("t p k m -> p t k m"))
            for t in range(2):
                nc.scalar.dma_start(dwk_sb[:, t, :], dwk_r[t])
                nc.scalar.dma_start(tkb_sb[:, t, :], tkb_r[t])
            nc.scalar.dma_start(fp[:, 0, :, :], fs_r[0])
            nc.gpsimd.dma_start(fp[:, 1, :, :], fs_r[1])
            nc.sync.dma_start(dwq_sb[:, :, :],
                              dwq_r.rearrange("t p k -> p t k"))
            nc.sync.dma_start(tqb_sb[:, :, :],
                              tqb_r.rearrange("t p k -> p t k"))
            nc.sync.dma_start(xp[:, :, :, :],
                              xs_r.rearrange("t p a b -> p t a b"))
            for t in range(2):
                nc.sync.dma_start(pwqT_sb[:, t, :], pwqT_r[t])
                nc.scalar.dma_start(pwkT_sb[:, t, :], pwkT_r[t])
                nc.gpsimd.dma_start(wvT_sb[:, t, :], wvT_r[t])
                nc.gpsimd.dma_start(bout_sb[:, t, :], bout_r[t])
            for t in range(4):
                nc.gpsimd.dma_start(woutT_sb[:, t, :], woutT_r[t])

            # v^T staging: [kv-chunk, head, 66] blocks; col 64 of each block
            # is the ones column (row-sum trick), col 65 unused padding.
            # (memset doesn't support f32r, so copy from an f32 ones tile.)
            vt_sb = actpool.tile([128, 8, HEADS, 66], a_dt)
            ones_sb = cpool.tile([128, 64], FP)
            nc.gpsimd.memset(ones_sb[:, :], 1.0)
            nc.vector.tensor_copy(
                vt_sb[:, :, :, 64:65],
                ones_sb[:, :].rearrange("p (a b c) -> p a b c", a=8, b=HEADS))

            tq = actpool.tile([128, 2, NQ], a_dt)
            tkv = actpool.tile([128, 2, NKV], a_dt)

            # ---------------- depthwise convs ----------------
            # All taps on DVE: GPSIMD's Pool ISA has no TensorScalarPtr
            # (per-partition scalar) op.  kv branch first — k/v gate more
            # PE work than q.
            def dw_conv(eng, src_ap, stride, n, wtile, btile, ct, dst,
                        half=None, epi_eng=None):
                # half: process only pixel rows [half] (kv branch) so the
                # first half of k/v unblocks attention chunks 0-3 early.
                acc = accpool.tile([128, n], FP, tag="dwacc")
                rows = 16 if stride == 1 else 16
                r0 = 0 if not half else (32 if stride == 1 else 32)
                av = acc[:, :].rearrange("p (a b) -> p a b", a=rows)
                for tap in range(9):
                    dy, dx = tap // 3, tap % 3
                    if stride == 1:
                        s = src_ap[:, ct, dy:dy + 16, dx:dx + 64]
                    else:
                        y0 = dy + half * 32
                        s = src_ap[:, ct, y0:y0 + 32:2, dx:dx + 64:2]
                    w = wtile[:, ct, tap:tap + 1]
                    if tap == 0:
                        eng.tensor_scalar(av, s, w, None, op0=OP.mult)
                    else:
                        eng.scalar_tensor_tensor(av, s, w, av,
                                                 op0=OP.mult, op1=OP.add)
                # t = relu(acc + bias); output dtype = a_dt
                if epi_eng is nc.scalar:
                    nc.scalar.activation(dst, acc[:, :], AF.Relu,
                                         bias=btile[:, ct, :])
                else:
                    nc.vector.tensor_scalar(dst, acc[:, :], btile[:, ct, :],
                                            0.0, op0=OP.add, op1=OP.max)

            def dwq_pe(ct):
                acc = psu.tile([128, 1024], FP, tag="uR")
                for half in range(2):
                    o = acc[:, half * 512:(half + 1) * 512]
                    for tap in range(9):
                        dy, dx = tap // 3, tap % 3
                        r0 = half * 8
                        rhs = xp[:, ct, dy + r0:dy + r0 + 8, dx:dx + 64]
                        nc.tensor.matmul(
                            o, dgq_sb[:, ct, tap, :], rhs,
                            start=(tap == 0), stop=(tap == 8))
                nc.scalar.activation(tq[:, ct, :], acc[:, :], AF.Relu,
                                     bias=tqb_sb[:, ct, :])

            q_sb = actpool.tile([128, 4, NQ], a_dt)
            k_sb = actpool.tile([128, 4, NKV], a_dt)

            def relu_epi(eng, out, in_):
                # relu from PSUM; on ScalarE (idle pre-attention, and relu
                # shares exp's ACT table set) or DVE (slack mid-attention)
                if eng is nc.scalar:
                    nc.scalar.activation(out, in_, AF.Relu)
                else:
                    eng.tensor_scalar(out, in_, 0.0, None, op0=OP.max)

            def pw_k_half(half, epi_eng):
                # k: [kc on partitions, kv pixels]  (column-parallel)
                for mt in range(4):
                    pk = ps.tile([128, 512], FP, tag="mm")
                    for ct in range(2):
                        nc.tensor.matmul(
                            pk[:, :],
                            _mm(pwkT_sb[:, ct, mt * 128:(mt + 1) * 128]),
                            _mm(tkv[:, ct, half * 512:(half + 1) * 512]),
                            start=(ct == 0), stop=(ct == 1))
                    relu_epi(epi_eng,
                             k_sb[:, mt, half * 512:(half + 1) * 512],
                             pk[:, :])

            def pw_v_range(kts, epi_eng):
                # v^T: [kv pixels on partitions, vc]  (row-parallel)
                for kt in kts:
                    pv = ps.tile([128, 1024], FP, tag="mm")
                    for ct in range(2):
                        nc.tensor.matmul(
                            pv[:, 0:512],
                            _mm(tkv[:, ct, kt * 128:(kt + 1) * 128]),
                            _mm(wvT_sb[:, ct, :]),
                            start=(ct == 0), stop=(ct == 1))
                    relu_epi(epi_eng, vt_sb[:, kt, :, 0:64],
                             pv[:, 0:512].rearrange("p (h d) -> p h d",
                                                    h=HEADS))

            def pw_q_all():
                # q: [qc on partitions, q pixels]
                for mt in range(4):
                    pq = ps.tile([128, 1024], FP, tag="mm")
                    for half in range(2):
                        o = pq[:, half * 512:(half + 1) * 512]
                        for ct in range(2):
                            nc.tensor.matmul(
                                o,
                                _mm(pwqT_sb[:, ct, mt * 128:(mt + 1) * 128]),
                                _mm(tq[:, ct, half * 512:(half + 1) * 512]),
                                start=(ct == 0), stop=(ct == 1))
                    relu_epi(nc.scalar, q_sb[:, mt, :], pq[:, :])

            # Emission order tuned for overlap: the DVE kv-tap chains are
            # the long serial pole at the start, so they lead; PE picks up
            # each downstream matmul group as its inputs land.
            for ct in range(2):
                dw_conv(nc.vector, fp, 2, NKV // 2, dwk_sb, tkb_sb, ct,
                        tkv[:, ct, 0:512], half=0, epi_eng=nc.scalar)
            if QUANT == "bf16":
                for ct in range(2):
                    dwq_pe(ct)
            else:
                for ct in range(2):
                    dw_conv(nc.vector, xp, 1, NQ, dwq_sb, tqb_sb, ct,
                            tq[:, ct, :], epi_eng=nc.scalar)
            pw_q_all()
            pw_k_half(0, nc.scalar)
            pw_v_range(range(0, 4), nc.scalar)
            # second tkv half: DVE taps run under the first attention pair;
            # their epilogues stay on DVE to keep ACT free for exp
            for ct in range(2):
                dw_conv(nc.vector, fp, 2, NKV // 2, dwk_sb, tkb_sb, ct,
                        tkv[:, ct, 512:1024], half=1, epi_eng=nc.vector)
            pw_k_half(1, nc.vector)
            pw_v_range(range(4, 8), nc.vector)

            # ---------------- attention ----------------
            # Heads processed in PAIRS with interleaved kv chunks: chunks
            # 0-3 of both heads only need the first tkv half, so they
            # overlap the DVE tap chains producing the second half.
            # After a head's P@v accumulation, uR is copied to SBUF at once
            # (frees its PSUM slot ~4us earlier than waiting for the whole
            # normalize chain), and to_out's K-accumulation is folded in
            # per pair (att rows of pair hp are exactly K-chunk hp).
            att_sb = actpool.tile([128, 4, NQ], a_dt)

            def normalize(h, u_sb, rrow):
                # att = u * (1/rowsum); rowsum = row 64 (ones-column trick).
                # (reciprocal_approx_fast only from partition 0 — reading it
                # at base partition 64 wedged the exec unit.)
                po = (h % 2) * 64
                pt = h // 2
                invr = spool.tile([1, 1024], FP, tag="invr")
                nc.vector.reciprocal_approx_fast(invr[:, :], rrow[:, :])
                invrb = spool.tile([64, 1024], FP, tag="invrb")
                nc.gpsimd.partition_broadcast(invrb[:, :], invr[:, :])
                nc.vector.tensor_tensor(att_sb[po:po + 64, pt, :],
                                        u_sb[0:64, :], invrb[:, :],
                                        op=OP.mult)

            for hp in range(HEADS // 2):
                heads = (2 * hp, 2 * hp + 1)
                uRs = [psu.tile([65, 1024], FP, tag="uR",
                                name=f"uR_{hp}_{j}") for j in range(2)]
                pend = [[], []]
                for c in range(8):
                    for j, h in enumerate(heads):
                        po = (h % 2) * 64
                        pt = h // 2
                        dp = ps.tile([128, 1024], FP, tag="mm")
                        for half in range(2):
                            nc.tensor.matmul(
                                dp[:, half * 512:(half + 1) * 512],
                                _mm(k_sb[po:po + 64, pt,
                                         c * 128:(c + 1) * 128]),
                                _mm(q_sb[po:po + 64, pt,
                                         half * 512:(half + 1) * 512]),
                                start=True, stop=True)
                        e = epool.tile([128, 1024], a_dt, tag="e")
                        nc.scalar.activation(e[:, :], dp[:, :], AF.Exp,
                                             scale=SCALE)
                        pend[j].append((c, e))
                        if len(pend[j]) > 1:
                            _emit_pv(nc, uRs[j], vt_sb, pend[j].pop(0), h)
                if hp < HEADS // 2 - 1:
                    for j, h in enumerate(heads):
                        _emit_pv(nc, uRs[j], vt_sb, pend[j].pop(0), h)
                        rrow = spool.tile([1, 1024], FP, tag="rrow",
                                          name=f"rrow_{hp}_{j}")
                        nc.vector.tensor_copy(rrow[:, :], uRs[j][64:65, :])
                        u_sb = uspool.tile([64, 1024], FP, tag="usb",
                                           name=f"usb_{hp}_{j}")
                        nc.vector.tensor_copy(u_sb[:, :], uRs[j][0:64, :])
                        normalize(h, u_sb, rrow)
                else:
                    # tail-optimized last pair: rrow extraction on ScalarE
                    # (idle after the final exp), then normalize multiplies
                    # interleaved with to_out column-half by column-half so
                    # the 16 to_out matmuls overlap the second-half mults.
                    invrbs = []
                    for j, h in enumerate(heads):
                        _emit_pv(nc, uRs[j], vt_sb, pend[j].pop(0), h)
                        rrow = spool.tile([1, 1024], FP, tag="rrow",
                                          name=f"rrowL_{j}")
                        nc.scalar.copy(rrow[:, :], uRs[j][64:65, :])
                        invr = spool.tile([1, 1024], FP, tag="invr",
                                          name=f"invrL_{j}")
                        nc.vector.reciprocal_approx_fast(invr[:, :],
                                                         rrow[:, :])
                        invrb = spool.tile([64, 1024], FP, tag="invrb",
                                           name=f"invrbL_{j}")
                        nc.gpsimd.partition_broadcast(invrb[:, :],
                                                      invr[:, :])
                        invrbs.append(invrb)
                    psos = []
                    for mt in range(2):
                        pso = ps.tile([128, 1024], FP, tag="mm",
                                      name=f"pso_{mt}")
                        psos.append(pso)
                    for half in range(2):
                        sl = slice(half * 512, (half + 1) * 512)
                        for j, h in enumerate(heads):
                            po = (h % 2) * 64
                            pt = h // 2
                            nc.vector.tensor_tensor(
                                att_sb[po:po + 64, pt, sl],
                                uRs[j][0:64, sl], invrbs[j][:, sl],
                                op=OP.mult)
                        for mt in range(2):
                            for ct in range(4):
                                nc.tensor.matmul(
                                    psos[mt][:, sl],
                                    _mm(woutT_sb[:, ct,
                                                 mt * 128:(mt + 1) * 128]),
                                    _mm(att_sb[:, ct, sl]),
                                    start=(ct == 0), stop=(ct == 3))

            # ---------------- output epilogue ----------------
            osb = actpool.tile([128, 2, NQ], FP)
            for mt in range(2):
                nc.vector.tensor_scalar(osb[:, mt, :], psos[mt][:, :],
                                        bout_sb[:, mt, :], 0.0,
                                        op0=OP.add, op1=OP.max)
                nc.sync.dma_start(out_r[mt], osb[:, mt, :])

    nc.compile()
    return nc


def _emit_pv(nc, uR, vt_sb, ce, h):
    c, e = ce
    for half in range(2):
        nc.tensor.matmul(uR[:, half * 512:(half + 1) * 512],
                         _mm(vt_sb[:, c, h, 0:65]),
                         _mm(e[:, half * 512:(half + 1) * 512]),
                         start=(c == 0), stop=(c == 7))


_NC_CACHE = {}


def _get_nc():
    key = QUANT
    if key not in _NC_CACHE:
        _NC_CACHE[key] = build_graph()
    return _NC_CACHE[key]


def _prep_shards(inputs):
    """Host-side sharding/layout prep. Returns in_maps for the 8 cores."""
    f32 = lambda a: np.ascontiguousarray(np.asarray(a, np.float32))
    w_np = np.float32 if QUANT != "bf16" else None

    def wcast(a):
        a = np.ascontiguousarray(np.asarray(a, np.float32))
        if QUANT == "bf16":
            import ml_dtypes
            a = a.astype(ml_dtypes.bfloat16)
        return a

    x = f32(inputs["x"])
    features = f32(inputs["features"])

    # fold BN into depthwise weights/bias
    sq = f32(inputs["bnq_g"]) / np.sqrt(f32(inputs["bnq_v"]) + EPS)
    sk = f32(inputs["bnk_g"]) / np.sqrt(f32(inputs["bnk_v"]) + EPS)
    dwq = f32(inputs["dw_q"])[:, 0] * sq[:, None, None]
    dwk = f32(inputs["dw_kv"])[:, 0] * sk[:, None, None]
    dwq = np.ascontiguousarray(dwq.reshape(DIM, 9))
    dwk = np.ascontiguousarray(dwk.reshape(DIM, 9))
    tqb = np.ascontiguousarray(
        (f32(inputs["bnq_b"]) - f32(inputs["bnq_m"]) * sq).reshape(DIM, 1))
    tkb = np.ascontiguousarray(
        (f32(inputs["bnk_b"]) - f32(inputs["bnk_m"]) * sk).reshape(DIM, 1))

    pw_q = f32(inputs["pw_q"])[:, :, 0, 0]       # (512, 256)
    pw_kv = f32(inputs["pw_kv"])[:, :, 0, 0]     # (1024, 256)
    w_out = f32(inputs["w_out"])[:, :, 0, 0]     # (256, 512)
    pwqT = wcast(pw_q.T)                          # (256, 512)
    pwkT = wcast(pw_kv[:INNER].T)                 # (256, 512)
    wvT = wcast(pw_kv[INNER:].T)                  # (256, 512)
    woutT = wcast(w_out.T)                        # (512, 256)
    bout = np.ascontiguousarray(f32(inputs["b_out"]).reshape(DIM, 1))

    dgq = None
    if QUANT == "bf16":
        # diagonal per-tap matrices for the PE q-branch depthwise conv
        import ml_dtypes
        d = np.zeros((DIM, 9, 128), np.float32)
        cc = np.arange(DIM)
        d[cc, :, cc % 128] = dwq
        dgq = np.ascontiguousarray(
            d.reshape(DIM, 9 * 128).astype(ml_dtypes.bfloat16))

    # zero-padded images
    xpad = np.zeros((B, DIM, HW_ + 2, HW_ + 2), np.float32)
    xpad[:, :, 1:-1, 1:-1] = x
    fpad = np.zeros((B, DIM, HW_ + 2, HW_ + 2), np.float32)
    fpad[:, :, 1:-1, 1:-1] = features

    if QUANT == "bf16":
        # images are stored/DMA'd in bf16 (the kernel computes in bf16)
        import ml_dtypes
        xpad = xpad.astype(ml_dtypes.bfloat16)
        fpad = fpad.astype(ml_dtypes.bfloat16)

    in_maps = []
    for c in range(N_CORES):
        b = c // CORES_PER_BATCH
        r0 = (c % CORES_PER_BATCH) * ROWS
        xs_c = np.ascontiguousarray(
            xpad[b, :, r0:r0 + ROWS + 2, :].reshape(DIM, 18 * 66))
        fs_c = np.ascontiguousarray(fpad[b].reshape(DIM, 66 * 66))
        m = {
            "xs": xs_c, "fs": fs_c,
            "dwq": dwq, "tqb": tqb, "dwk": dwk, "tkb": tkb,
            "pwqT": pwqT, "pwkT": pwkT, "wvT": wvT,
            "woutT": woutT, "bout": bout,
        }
        if dgq is not None:
            m["dgq"] = dgq
        in_maps.append(m)
    return in_maps


def kernel(**inputs):
    nc = _get_nc()
    in_maps = _prep_shards(inputs)
    trace = os.environ.get("KERNEL_TRACE", "0") == "1"
    res = run_bass_kernel_spmd(nc, in_maps, core_ids=list(range(N_CORES)),
                               trace=trace)
    if trace:
        kernel.last_exec_time_ns = res.exec_time_ns
        kernel.last_results = res
    out = np.zeros((B, DIM, HW_, HW_), np.float32)
    for c in range(N_CORES):
        b = c // CORES_PER_BATCH
        r0 = (c % CORES_PER_BATCH) * ROWS
        out[b, :, r0:r0 + ROWS, :] = res.results[c]["out"].reshape(
            DIM, ROWS, HW_)
    return out


if __name__ == "__main__":
    nc = build_graph()
    print("graph built + compiled OK")
